# revision 1
# baseline (speedup 1.0000x reference)
"""Trainium2 Bass kernel v2 for nn_MultiHeadAttention_81655918232272.

Reference semantics:
    q = (x @ Wq).reshape(B, N, H, Dh)   # H=16 heads, Dh=64 (biases zero)
    scores = einsum("bnhd,bngd->bnhg", q, k)   # per-token 16x16 head-mixing
    ctx = softmax(scores, -1) @ v ; out = ctx.reshape(.., 1024) @ Wo

Design (per core: 4096 tokens, data-parallel over batch), all bf16 on PE:
  per 128-token tile:
    - forward QKV projections [128t x 1024]
    - per-head PE transposes of q,k -> [64d, 128t] psum (partition 0-63),
      DVE strided copies build G_q,G_k [64d, (t8 h)-interleaved] operands
    - scores: 16 group matmuls K=64 -> full [128,128] cross tiles;
      exp on ACT (bf16 holds e^46), block-diag mask multiply on DVE
    - V carries a fused ones-column: ctx matmul emits [128 (t8 h), 65]
      = unnormalized ctx + softmax denominator in one pass
    - coarse SBUF->SBUF DMA shuffles (130B runs) for vt and ctx-return;
      normalize via per-partition reciprocal + tensor_scalar muls
    - PE transpose ctx -> forward Wo projection -> out (fp32)
  No DRAM staging, no fine-grained descriptors, no DMA transposes.
"""

import numpy as np

H = 16
DH = 64
DIM = 1024
B, N = 32, 1024
NCORES = 8
BPC = B // NCORES          # batches per core
T = BPC * N                # tokens per core (4096)
NTILE = T // 128           # 128-token tiles per core (32)

_CACHE = {}


def _build(ntile=NTILE, debug=False):
    import concourse.bass as bass  # noqa: F401
    import concourse.mybir as mybir
    import concourse.tile as tile
    from concourse import bacc
    from concourse.masks import make_identity
    from contextlib import ExitStack

    bf16, fp32 = mybir.dt.bfloat16, mybir.dt.float32
    fp16 = mybir.dt.float16
    Exp = mybir.ActivationFunctionType.Exp
    Tl = 128 * ntile

    nc = bacc.Bacc(None, target_bir_lowering=False, debug=debug)

    with tile.TileContext(nc) as tc, ExitStack() as ctx:
        dram = ctx.enter_context(tc.tile_pool(name="dram", bufs=1, space="DRAM"))
        const = ctx.enter_context(tc.tile_pool(name="const", bufs=1))
        sbA = ctx.enter_context(tc.tile_pool(name="sbA", bufs=2))
        sbB = ctx.enter_context(tc.tile_pool(name="sbB", bufs=2))
        dstage = ctx.enter_context(tc.tile_pool(name="dstage", bufs=2, space="DRAM"))
        gemm_ps = ctx.enter_context(tc.tile_pool(name="gemm_ps", bufs=4, space="PSUM"))
        tr_ps = ctx.enter_context(tc.tile_pool(name="tr_ps", bufs=2, space="PSUM"))
        s_ps = ctx.enter_context(tc.tile_pool(name="s_ps", bufs=2, space="PSUM"))

        # ---- DRAM I/O ----
        xT_d = dram.tile([DIM, Tl], fp16, kind="ExternalInput")
        w_d = {}
        for wname in ("wq", "wk", "wv", "wo"):
            w_d[wname] = dram.tile([DIM, DIM], fp16, kind="ExternalInput",
                                   name=f"{wname}_d")
        mask_d = dram.tile([128, 512], bf16, kind="ExternalInput")
        out_d = dram.tile([Tl, DIM], fp32, kind="ExternalOutput")

        # ---- resident SBUF ----
        w_sb = {}
        for wname in ("wq", "wk", "wv", "wo"):
            wt = const.tile([128, 8 * DIM], fp16, tag=f"w_{wname}", name=f"w_{wname}_sb")
            for kt in range(8):
                nc.sync.dma_start(wt[:, DIM * kt:DIM * (kt + 1)],
                                  w_d[wname][128 * kt:128 * (kt + 1), :])
            w_sb[wname] = wt
        mask_sb = const.tile([128, 512], bf16)
        nc.sync.dma_start(mask_sb[:], mask_d[:])
        ident = const.tile([128, 128], fp16)
        make_identity(nc, ident[:])

        def stage_a(i):
            """QKV projections + q/k transposes + G operands + vt for tile i."""
            t0 = 128 * i
            xt = sbA.tile([128, 8 * 128], fp16, tag="xt", name="xt")
            nc.sync.dma_start(
                xt[:].rearrange("f (kt t) -> f kt t", t=128),
                xT_d[:, t0:t0 + 128].rearrange("(kt f) t -> f kt t", f=128))

            q16 = sbA.tile([128, DIM], fp16, tag="q16", name="q16")
            k16 = sbA.tile([128, DIM], fp16, tag="k16", name="k16")
            v16 = sbA.tile([128, DIM], bf16, tag="v16", name="v16")
            for wname, dst in (("wq", q16), ("wk", k16), ("wv", v16)):
                pss = [gemm_ps.tile([128, 512], fp32, tag="gemm", name="gemm_ps")
                       for _ in range(2)]
                for kt in range(8):
                    for n in range(2):
                        nc.tensor.matmul(
                            pss[n][:],
                            xt[:, 128 * kt:128 * (kt + 1)],
                            w_sb[wname][:, DIM * kt + 512 * n:DIM * kt + 512 * (n + 1)],
                            start=(kt == 0), stop=(kt == 7))
                for n in range(2):
                    nc.scalar.copy(dst[:, 512 * n:512 * (n + 1)], pss[n][:])

            Gq = sbA.tile([64, 16 * 128], fp16, tag="Gq", name="Gq")
            Gk = sbA.tile([64, 16 * 128], fp16, tag="Gk", name="Gk")
            for src, G in ((q16, Gq), (k16, Gk)):
                for quad in range(4):
                    trp = tr_ps.tile([128, 512], fp16, tag="trp", name="trp")
                    for hh in range(4):
                        h = 4 * quad + hh
                        nc.tensor.transpose(trp[0:64, 128 * hh:128 * (hh + 1)],
                                            src[:, DH * h:DH * (h + 1)], ident[:])
                    nc.vector.tensor_copy(
                        G[:].rearrange("d (t h) -> d t h", h=H)[:, :, 4 * quad:4 * (quad + 1)],
                        trp[0:64, :].rearrange("d (hh t) -> d t hh", t=128))

            v_dr = dstage.tile([128, DIM], bf16, tag="v_dr", name="v_dr")
            nc.sync.dma_start(v_dr[:], v16[:])
            vt = sbA.tile([128, 16 * 65], bf16, tag="vt", name="vt")
            nc.vector.memset(vt[:].rearrange("p (g dd) -> p g dd", dd=65)[:, :, 64], 1.0)
            for a in range(8):
                nc.gpsimd.dma_start(
                    vt[16 * a:16 * (a + 1), :].rearrange("g (grp dd) -> g grp dd", dd=65)[:, :, 0:DH],
                    v_dr[:].rearrange("(grp a) (g d) -> a g grp d", a=8, d=DH)[a])
            return dict(Gq=Gq, Gk=Gk, vt=vt)

        def stage_b1a(i, st):
            """Scores + exp + mask for tile i."""
            Gq, Gk = st["Gq"], st["Gk"]
            E = sbB.tile([128, 16 * 128], bf16, tag="E", name="E")
            for c in range(4):
                sp = s_ps.tile([128, 512], fp32, tag="s", name="s_ps")
                for g in range(4):
                    grp = 4 * c + g
                    nc.tensor.matmul(sp[:, 128 * g:128 * (g + 1)],
                                     Gk[:, 128 * grp:128 * (grp + 1)],
                                     Gq[:, 128 * grp:128 * (grp + 1)],
                                     start=True, stop=True)
                tmp = sbB.tile([128, 512], bf16, tag="etmp", name="etmp")
                nc.scalar.activation(tmp[:], sp[:], Exp)
                nc.vector.tensor_mul(E[:, 512 * c:512 * (c + 1)], tmp[:], mask_sb[:])
            return E

        def stage_b1b(i, st, E):
            """ctx matmuls + return shuffle for tile i."""
            vt = st["vt"]
            ctxu = sbB.tile([128, 16 * 65], bf16, tag="ctxu", name="ctxu")
            for c in range(4):
                cp = s_ps.tile([128, 512], fp32, tag="s", name="ctx_ps")
                for g in range(4):
                    grp = 4 * c + g
                    nc.tensor.matmul(cp[:, 65 * g:65 * (g + 1)],
                                     E[:, 128 * grp:128 * (grp + 1)],
                                     vt[:, 65 * grp:65 * (grp + 1)],
                                     start=True, stop=True)
                nc.scalar.copy(ctxu[:, 260 * c:260 * (c + 1)], cp[:, 0:260])

            cu_dr = dstage.tile([128, 16 * 65], bf16, tag="cu_dr", name="cu_dr")
            for a in range(8):
                nc.gpsimd.dma_start(
                    cu_dr[:].rearrange("(grp aa) (h dd) -> aa h grp dd", aa=8, dd=65)[a],
                    ctxu[16 * a:16 * (a + 1), :].rearrange("h (grp dd) -> h grp dd", dd=65))
            ctxf = sbB.tile([128, 16 * 65], bf16, tag="ctxf", name="ctxf")
            nc.sync.dma_start(ctxf[:], cu_dr[:])
            return ctxf

        def stage_b2(i, ctxf):
            """Normalize + ctx transpose + Wo projection + store for tile i."""
            rcp = sbB.tile([128, 16], fp32, tag="rcp", name="rcp")
            nc.vector.reciprocal(
                rcp[:], ctxf[:].rearrange("t (h dd) -> t h dd", dd=65)[:, :, 64])
            ctxn = sbB.tile([128, DIM], fp16, tag="ctxn", name="ctxn")
            for h in range(16):
                nc.vector.tensor_scalar_mul(
                    ctxn[:, DH * h:DH * (h + 1)],
                    ctxf[:].rearrange("t (h dd) -> t h dd", dd=65)[:, h, 0:DH],
                    rcp[:, h:h + 1])

            ctxT = sbB.tile([128, DIM], fp16, tag="ctxT", name="ctxT")
            for c in range(2):
                tp = tr_ps.tile([128, 512], fp16, tag="trp", name="ctxT_ps")
                for j in range(4):
                    cc = 4 * c + j
                    nc.tensor.transpose(tp[:, 128 * j:128 * (j + 1)],
                                        ctxn[:, 128 * cc:128 * (cc + 1)], ident[:])
                nc.vector.tensor_copy(ctxT[:, 512 * c:512 * (c + 1)], tp[:])

            out_sb = sbB.tile([128, DIM], fp32, tag="out_sb", name="out_sb")
            pss = [gemm_ps.tile([128, 512], fp32, tag="gemm", name="gemm_ps2")
                   for _ in range(2)]
            for b in range(8):
                for n in range(2):
                    nc.tensor.matmul(
                        pss[n][:], ctxT[:, 128 * b:128 * (b + 1)],
                        w_sb["wo"][:, DIM * b + 512 * n:DIM * b + 512 * (n + 1)],
                        start=(b == 0), stop=(b == 7))
            for n in range(2):
                nc.scalar.copy(out_sb[:, 512 * n:512 * (n + 1)], pss[n][:])
            nc.sync.dma_start(out_d[128 * i:128 * (i + 1), :], out_sb[:])

        # 4-segment software pipeline: b2(i-2) | b1a(i) | a(i+1) | b1b(i)
        states = {0: stage_a(0)}
        Es, ctxfs = {}, {}
        for i in range(ntile + 2):
            if i - 2 >= 0:
                stage_b2(i - 2, ctxfs.pop(i - 2))
            if i < ntile:
                Es[i] = stage_b1a(i, states[i])
            if i + 1 < ntile:
                states[i + 1] = stage_a(i + 1)
            if i < ntile:
                ctxfs[i] = stage_b1b(i, states.pop(i), Es.pop(i))

    nc.compile()
    return nc


def _make_mask():
    m = np.kron(np.eye(8, dtype=np.float32), np.ones((16, 16), np.float32))
    return np.tile(m, (1, 4))  # [128, 512]


def _prep_inputs(x, Wq, Wk, Wv, Wo, ntile=NTILE):
    import ml_dtypes
    bf = ml_dtypes.bfloat16
    Tl = 128 * ntile
    w16 = {
        "wq": np.ascontiguousarray(Wq.astype(np.float16)),
        "wk": np.ascontiguousarray(Wk.astype(np.float16)),
        "wv": np.ascontiguousarray(Wv.astype(np.float16)),
        "wo": np.ascontiguousarray(Wo.astype(np.float16)),
    }
    mask = _make_mask().astype(bf)
    ncores = x.shape[0] * x.shape[1] // Tl
    in_maps = []
    for c in range(ncores):
        shard = np.asarray(x).reshape(-1, DIM)[Tl * c:Tl * (c + 1)]
        xT = np.ascontiguousarray(shard.T.astype(np.float16))
        m = {"xT_d": xT, "mask_d": mask}
        for k, v in w16.items():
            m[k + "_d"] = v
        in_maps.append(m)
    return in_maps


def _resolve_names(nc):
    import concourse.mybir as mybir
    in_names, out_name = [], None
    for alloc in nc.m.functions[0].allocations:
        if not isinstance(alloc, mybir.MemoryLocationSet):
            continue
        if alloc.kind == "ExternalInput":
            in_names.append(alloc.memorylocations[0].name)
        elif alloc.kind == "ExternalOutput":
            out_name = alloc.memorylocations[0].name
    return in_names, out_name


def _install_ntff_hook():
    import sys, types
    try:
        from antenv.axon_hooks import get_axon_ntff_profile_hook  # noqa: F401
        return
    except ImportError:
        pass
    try:
        from trn_agent_boot.trn_boot import _ntff_profile_via_ctypes
        hook = _ntff_profile_via_ctypes('/opt/axon/libaxon_pjrt.so')
    except Exception:
        hook = None
    mod = types.ModuleType('antenv.axon_hooks')
    mod._hook = hook
    mod.get_axon_ntff_profile_hook = lambda: mod._hook
    mod.set_axon_ntff_profile_hook = lambda h: setattr(mod, '_hook', h)
    sys.modules['antenv.axon_hooks'] = mod


def kernel(x, Wq, bq, Wk, bk, Wv, bv, Wo, bo, trace=False):
    from concourse.bass_utils import run_bass_kernel_spmd

    if trace:
        _install_ntff_hook()

    if "nc" not in _CACHE:
        _CACHE["nc"] = _build()
    nc = _CACHE["nc"]

    in_names, out_name = _resolve_names(nc)

    def resolve(logical):
        for nm in in_names:
            if nm == logical or nm.startswith(logical + "_") or nm.startswith(logical):
                return nm
        raise KeyError(f"no DRAM tensor matching {logical}: {in_names}")

    raw_maps = _prep_inputs(np.asarray(x), np.asarray(Wq), np.asarray(Wk),
                            np.asarray(Wv), np.asarray(Wo))
    in_maps = [{resolve(k): v for k, v in m.items()} for m in raw_maps]

    res = run_bass_kernel_spmd(nc, in_maps, core_ids=list(range(NCORES)),
                               trace=trace)
    outs = [res.results[c][out_name].reshape(BPC, N, DIM) for c in range(NCORES)]
    full = np.concatenate(outs, axis=0).astype(np.float32)
    if trace:
        kernel.last_exec_time_ns = res.exec_time_ns
    return full

